# revision 8
# baseline (speedup 1.0000x reference)
"""Multi-head attention Bass/Tile kernel for TRN2, sharded 8 ways.

Sharding: core c handles batch b = c//2 and heads half = c%2 (8 of 16 heads).
Each core computes, for its batch and its 8 heads:
  q/k/v projections -> scoresT = K @ Q^T (per head, [t, s] layout) -> exp ->
  per-head PV: 16 accumulating K=128 matmuls into a [65, 512] psum tile,
  where lhsT = v' (64 value cols + a ones column) so psum row 64 ends up
  holding the softmax denominator sum_t exp for free -> reciprocal of that
  row (psum[64:65] -> sbuf[0:1], HW-verified) -> gpsimd partition_broadcast
  -> one DVE mul into catT -> partial output projection. Host sums the two
  partials per batch and adds the bias.

v2 vs the earlier quad-PV design: the DVE exp-sum chain (15 adds/unit), the
ones-matmul denominators and the psum t-half folds are all gone; both heads
of a pair keep all data on partitions 0:64 (no partition-shifting DVE ops,
which corrupt on HW). catT is [64, 8*512] (one 512-col block per head) and
the output projection contracts i in 8 chunks of K=64; Wout is reordered on
the host to match.

Layout choices (no transposes anywhere):
  xT     [D, S]  : host-pretransposed activations (d on partitions)
  wq/wk  [D, H*dk] : lhsT layout for qT/kT = W^T @ xT
  wv     [D, H*dk] : rhs layout for v = xT^T @ wv  ([t, vdim], natural)
  kT     [H*dk, S]: j on partitions -> head-pair p lives in 128-row chunk p
  scoresT[t, s]   : lhsT=kT [j,t] rows 0:64 / 64:128 (concurrent pair)
  v'     [t, h*65+0:64]=v, col h*65+64 = ones (memset preset)
  pv     [65, 512] psum: rows 0:64 = sum_t exp*v, row 64 = sum_t exp
  catT   [64, 8*S_BLK]: block 2*hp+head, rows = dv
  out    [s, o]   : lhsT=catT block [64, s-chunk] (K=64), rhs=wo block

HW pitfalls baked in (learned on-device):
  - no partition-shifting DVE copies; the only cross-partition moves are
    InstReciprocal psum[64:65]->sbuf[0:1] (verified) and gpsimd
    partition_broadcast
  - matmul free dim capped at 512; psum tiles are bank-aligned
"""

from contextlib import ExitStack
from dataclasses import dataclass

import numpy as np
import ml_dtypes

import concourse.bass as bass  # noqa: F401
import concourse.tile as tile
from concourse import bacc, mybir


@dataclass
class Cfg:
    D: int = 1024      # model dim
    S: int = 2048      # sequence length (queries == keys)
    HL: int = 8        # heads per core
    DK: int = 64       # head dim
    S_BLK: int = 512   # query block (matmul free dim)
    T_BLK: int = 512   # t block in projection phase

    @property
    def DC(self):
        return self.D // 128

    @property
    def NSB(self):
        return self.S // self.S_BLK

    @property
    def TBn(self):
        return self.S // self.T_BLK

    @property
    def TCn(self):
        return self.S // 128

    @property
    def JW(self):
        return self.HL * self.DK

    @property
    def JC(self):
        return self.JW // 128

    @property
    def VW(self):
        return self.DK + 1

    @property
    def OB(self):
        return min(512, self.D)


DT_NP = {
    mybir.dt.bfloat16: ml_dtypes.bfloat16,
    mybir.dt.float32: np.float32,
    mybir.dt.float32r: np.float32,
}


def build_nc(cfg: Cfg, DT=mybir.dt.bfloat16, num_devices: int = 8):
    c = cfg
    f32 = mybir.dt.float32
    EXPDT = DT if DT == mybir.dt.bfloat16 else f32
    SCALE = 1.0 / float(np.sqrt(c.DK))
    nc = bacc.Bacc("TRN2", target_bir_lowering=False, debug=False,
                   num_devices=num_devices)

    xqT = nc.dram_tensor("xqT", [c.D, c.S], DT, kind="ExternalInput").ap()
    xkT = nc.dram_tensor("xkT", [c.D, c.S], DT, kind="ExternalInput").ap()
    xvT = nc.dram_tensor("xvT", [c.D, c.S], DT, kind="ExternalInput").ap()
    wq_d = nc.dram_tensor("wq", [c.D, c.JW], DT, kind="ExternalInput").ap()
    wk_d = nc.dram_tensor("wk", [c.D, c.JW], DT, kind="ExternalInput").ap()
    wv_d = nc.dram_tensor("wv", [c.D, c.JW], DT, kind="ExternalInput").ap()
    # wo: [64 dv rows, (2*hp+head) blocks of D] (host-reordered)
    wo_d = nc.dram_tensor("woutT", [c.DK, 2 * c.JC * c.D], DT,
                          kind="ExternalInput").ap()
    out_d = nc.dram_tensor("out", [c.S, c.D], f32, kind="ExternalOutput").ap()

    from collections import deque

    with tile.TileContext(nc) as tc, ExitStack() as es:
        wpool = es.enter_context(tc.tile_pool(name="weights", bufs=1))
        kvpool = es.enter_context(tc.tile_pool(name="kv", bufs=1))
        xkpool = es.enter_context(tc.tile_pool(name="xk", bufs=4))
        xqpool = es.enter_context(tc.tile_pool(name="xq", bufs=2))
        xvpool = es.enter_context(tc.tile_pool(name="xv", bufs=2))
        qpool = es.enter_context(tc.tile_pool(name="q", bufs=2))
        epool = es.enter_context(tc.tile_pool(name="exp", bufs=18))
        cpool = es.enter_context(tc.tile_pool(name="cat", bufs=2))
        opool = es.enter_context(tc.tile_pool(name="o", bufs=1))
        rpool = es.enter_context(tc.tile_pool(name="r", bufs=1))
        pspool = es.enter_context(tc.tile_pool(name="ps", bufs=2, space="PSUM"))
        pvpool = es.enter_context(tc.tile_pool(name="pv", bufs=2, space="PSUM"))
        fppool = es.enter_context(tc.tile_pool(name="fp", bufs=2, space="PSUM"))

        def load_w_dmaj(dram, width, tag):
            t = wpool.tile([128, c.DC * width], DT, tag=tag, name=tag)
            for d in range(c.DC):
                eng = nc.sync if d % 2 == 0 else nc.gpsimd
                eng.dma_start(t[:, d * width:(d + 1) * width],
                              dram[d * 128:(d + 1) * 128, :])
            return t

        def load_x(pool, dram, blk, width, name):
            t = pool.tile([128, c.DC * width], DT, tag="x", name=name)
            for d in range(c.DC):
                eng = nc.sync if d % 2 == 0 else nc.gpsimd
                eng.dma_start(
                    t[:, d * width:(d + 1) * width],
                    dram[d * 128:(d + 1) * 128, blk * width:(blk + 1) * width])
            return t

        NT = c.T_BLK

        # ---- head DMAs: wk + xk0 lead (kT tb0/jc0 inline), then wq/xq0 for
        # qT(0) jc0; the rest arrives while the pipeline runs.
        wk_sb = load_w_dmaj(wk_d, c.JW, "wk")
        kT_sb = kvpool.tile([128, c.JC * c.S], DT, tag="kT", name="kT")
        xk_tiles = [load_x(xkpool, xkT, 0, NT, "xk0")]
        wq_sb = load_w_dmaj(wq_d, c.JW, "wq")
        xq_tiles = {0: load_x(xqpool, xqT, 0, c.S_BLK, "xq0")}
        xk_tiles += [load_x(xkpool, xkT, tb, NT, f"xk{tb}")
                     for tb in range(1, c.TBn)]
        wv_sb = load_w_dmaj(wv_d, c.JW, "wv")
        v_sb = kvpool.tile([128, c.TCn * c.HL * c.VW], DT, tag="v", name="v")
        nc.gpsimd.memset(v_sb[:], 1.0)  # ones columns preset
        xv_tiles = {0: load_x(xvpool, xvT, 0, NT, "xv0"),
                    1: load_x(xvpool, xvT, 1, NT, "xv1")}
        xq_tiles[1] = load_x(xqpool, xqT, 1, c.S_BLK, "xq1")
        wo_sb = wpool.tile([c.DK, 2 * c.JC * c.D], DT, tag="wo", name="wo")
        for ic in range(2 * c.JC):
            eng = nc.sync if ic % 2 == 0 else nc.gpsimd
            eng.dma_start(wo_sb[:, ic * c.D:(ic + 1) * c.D],
                          wo_d[:, ic * c.D:(ic + 1) * c.D])

        def kT_ops(tb, jc):
            """8 MM closures computing kT chunk jc for t-block tb."""
            box = {}

            def mk(d):
                def op():
                    if d == 0:
                        box["ps"] = fppool.tile([128, NT], f32, tag="fp",
                                                name=f"psk{tb}_{jc}")
                    nc.tensor.matmul(
                        box["ps"][:],
                        wk_sb[:, d * c.JW + jc * 128: d * c.JW + (jc + 1) * 128],
                        xk_tiles[tb][:, d * NT:(d + 1) * NT],
                        start=(d == 0), stop=(d == c.DC - 1))
                    if d == c.DC - 1:
                        nc.vector.tensor_copy(
                            kT_sb[:, jc * c.S + tb * NT: jc * c.S + (tb + 1) * NT],
                            box["ps"][:])
                return op
            return [mk(d) for d in range(c.DC)]

        # count of v' chunk-groups whose SBUF copy has been EMITTED — PV pops
        # for sb-0 units must not overtake this.
        v_done = [0]

        def v_ops(tb):
            """v' projection closures for t-block tb (+ trailing xv prefetch)."""
            ops = []
            for tt in range(NT // 128):
                g = tb * (NT // 128) + tt
                box = {}

                def mk(d, g=g, tt=tt, tb=tb, box=box):
                    def op():
                        if d == 0:
                            box["ps"] = fppool.tile([128, c.JW], f32, tag="fp",
                                                    name=f"psv{g}")
                        nc.tensor.matmul(
                            box["ps"][:],
                            xv_tiles[tb][:, d * NT + tt * 128:
                                         d * NT + (tt + 1) * 128],
                            wv_sb[:, d * c.JW:(d + 1) * c.JW],
                            start=(d == 0), stop=(d == c.DC - 1))
                        if d == c.DC - 1:
                            dst = v_sb[:, g * c.HL * c.VW:(g + 1) * c.HL * c.VW]
                            dst3 = dst.rearrange("p (h w) -> p h w",
                                                 w=c.VW)[:, :, 0:c.DK]
                            src3 = box["ps"][:].rearrange("p (h w) -> p h w",
                                                          w=c.DK)
                            nc.vector.tensor_copy(dst3, src3)
                            v_done[0] = g + 1
                    return op
                ops += [mk(d) for d in range(c.DC)]
            if tb + 2 < c.TBn:
                def pf(tb=tb):
                    xv_tiles[tb + 2] = load_x(xvpool, xvT, tb + 2, NT,
                                              f"xv{tb + 2}")
                ops.append(pf)
            return ops

        def emit_qT_mms(sb, xq, qT):
            """32 MM closures (jc-major); last per jc copies psum -> qT chunk
            jc (head A rows 0:64, head B rows 64:128 — natural layout)."""
            ops = []
            psq_box = {}

            def mk(jc, d):
                def op():
                    if d == 0:
                        psq_box[jc] = fppool.tile([128, c.S_BLK], f32, tag="fp",
                                                  name=f"psq{sb}_{jc}")
                    nc.tensor.matmul(
                        psq_box[jc][:],
                        wq_sb[:, d * c.JW + jc * 128: d * c.JW + (jc + 1) * 128],
                        xq[:, d * c.S_BLK:(d + 1) * c.S_BLK],
                        start=(d == 0), stop=(d == c.DC - 1))
                    if d == c.DC - 1:
                        nc.vector.tensor_copy(
                            qT[:, jc * c.S_BLK:(jc + 1) * c.S_BLK],
                            psq_box[jc][:])
                return op
            for jc in range(c.JC):
                for d in range(c.DC):
                    ops.append(mk(jc, d))
            return ops

        def emit_outproj_mms(sb, catT):
            """Closures: per (sc, oc): 8 K=64 ic-MMs into a 1-bank psum, then
            copy + DMA out."""
            ops = []
            po_box = {}
            NIC = 2 * c.JC

            def mk(sc, oc, ic):
                def op():
                    if ic == 0:
                        po_box[(sc, oc)] = fppool.tile(
                            [128, c.OB], f32, tag="fp", name=f"po{sb}_{sc}_{oc}")
                    po = po_box[(sc, oc)]
                    nc.tensor.matmul(
                        po[:],
                        catT[0:c.DK,
                             ic * c.S_BLK + sc * 128:
                             ic * c.S_BLK + (sc + 1) * 128],
                        wo_sb[0:c.DK, ic * c.D + oc * c.OB:
                              ic * c.D + (oc + 1) * c.OB],
                        start=(ic == 0), stop=(ic == NIC - 1))
                    if ic == NIC - 1:
                        ot = opool.tile([128, c.OB], f32, tag="ot",
                                        name=f"ot{sb}_{sc}_{oc}")
                        nc.vector.tensor_copy(ot[:], po[:])
                        eng = nc.sync if (sc + oc) % 2 == 0 else nc.gpsimd
                        eng.dma_start(
                            out_d[sb * c.S_BLK + sc * 128:
                                  sb * c.S_BLK + (sc + 1) * 128,
                                  oc * c.OB:(oc + 1) * c.OB],
                            ot[:])
                return op
            for sc in range(c.S_BLK // 128):
                for oc in range(c.D // c.OB):
                    for ic in range(NIC):
                        ops.append(mk(sc, oc, ic))
            return ops

        # ---- slim prologue: only what unit (0,0) chunk 0 needs ----
        for op in kT_ops(0, 0):
            op()
        qT_tiles = {0: qpool.tile([128, c.JC * c.S_BLK], DT, tag="qT",
                                  name="qT0")}
        q0 = emit_qT_mms(0, xq_tiles[0], qT_tiles[0])
        for op in q0[0:8]:
            op()

        # ---- units + filler lists ----
        units = [(sb, hp) for sb in range(c.NSB) for hp in range(c.JC)]
        fillers = [[] for _ in units]
        # deadlines: kT(tb,jc) by chunk 4*tb of unit (0,jc); q0 jc by end of
        # unit (0,jc-1); v groups gated by v_done + deep PV lag.
        fillers[0] += kT_ops(1, 0) + kT_ops(2, 0) + kT_ops(3, 0) \
            + v_ops(0) + v_ops(1) + q0[8:16] + kT_ops(0, 1)
        fillers[1] += kT_ops(1, 1) + kT_ops(2, 1) + kT_ops(3, 1) \
            + v_ops(2) + v_ops(3) + q0[16:24] + kT_ops(0, 2)
        fillers[2] += kT_ops(1, 2) + kT_ops(2, 2) + kT_ops(3, 2) \
            + q0[24:32] + kT_ops(0, 3)
        fillers[3] += kT_ops(1, 3) + kT_ops(2, 3) + kT_ops(3, 3)

        cat_tiles = {}

        # ---- lagged-PV queue machinery (2 ops per t-chunk now) ----
        pvq = deque()        # (key, op, islast, need_v)
        stage_runs = {}      # key -> closure(cur_idx)
        pending_tail = []    # ops deferred past the last unit (drain)
        PVLAG = 32           # one full unit behind (2 ops/chunk)
        PVLAG0 = 32          # deep hold while sb-0's v' fillers land

        def emit_stage(sb, hp, catT, pv_state, cur_idx):
            """Normalize both heads from their pv psum tiles: reciprocal of
            the ones-row (psum[64:65]->sbuf[0:1], verified on HW), broadcast,
            one mul per head into catT block 2*hp+head. All ops stay on
            partitions 0:64 — no partition shifts."""
            for h in range(2):
                pv = pv_state["pvA" if h == 0 else "pvB"]
                rti = rpool.tile([1, c.S_BLK], f32, tag=f"rti{h}",
                                 name=f"rti{h}_{sb}_{hp}")
                nc.vector.reciprocal(rti[:], pv[64:65, :])
                rb = rpool.tile([128, c.S_BLK], f32, tag=f"rb{h}",
                                name=f"rb{h}_{sb}_{hp}")
                nc.gpsimd.partition_broadcast(rb[:], rti[:])
                blk = 2 * hp + h
                nc.vector.tensor_mul(
                    catT[0:c.DK, blk * c.S_BLK:(blk + 1) * c.S_BLK],
                    pv[0:c.DK, :], rb[0:c.DK, :])
            if hp == c.JC - 1:
                oops = emit_outproj_mms(sb, catT)
                splits = [(0, 24), (24, 48), (48, 64)]
                for j, (lo, hi) in enumerate(splits):
                    tgt = cur_idx + 1 + j
                    if tgt < len(units):
                        fillers[tgt] += oops[lo:hi]
                    else:
                        pending_tail.extend(oops[lo:hi])

        def make_pv_ops(key, sb, hp, t, es_tile, pv_state):
            """2 accumulating K=128 matmuls (head A, head B) per t-chunk.
            lhsT = v' [t-rows, 64 values + ones col] -> out [65, 512] with
            row 64 = sum_t exp. start clears the bank at t==0."""
            W = c.HL * c.VW
            first = (t == 0)
            last = (t == c.TCn - 1)

            def mk(h):
                vsl = slice(t * W + (2 * hp + h) * c.VW,
                            t * W + (2 * hp + h) * c.VW + c.VW)
                ecol = slice(h * c.S_BLK, (h + 1) * c.S_BLK)
                pkey = "pvA" if h == 0 else "pvB"

                def op():
                    if first:
                        pv_state[pkey] = pvpool.tile(
                            [128, c.S_BLK], f32, tag="pv",
                            name=f"pv{h}_{sb}_{hp}")
                    nc.tensor.matmul(
                        pv_state[pkey][0:c.VW, :],
                        v_sb[:, vsl], es_tile[:, ecol],
                        start=first, stop=last)
                return op
            need_v = t + 1 if sb == 0 else 0
            return [(key, mk(0), False, need_v),
                    (key, mk(1), last, need_v)]

        def pop_pv(n, cur_idx):
            for _ in range(n):
                if not pvq:
                    return
                if pvq[0][3] > v_done[0]:
                    return  # its v' chunk copy not yet emitted
                key, op, islast, _ = pvq.popleft()
                op()
                if islast:
                    stage_runs.pop(key)(cur_idx)

        # ---- main pipeline over units ----
        for idx, (sb, hp) in enumerate(units):
            if hp == 0:
                cat_tiles[sb] = cpool.tile([c.DK, 2 * c.JC * c.S_BLK], DT,
                                           tag="cat", name=f"catT{sb}")
            if sb == 0 and hp == 3 and c.NSB > 2:
                xq_tiles[2] = load_x(xqpool, xqT, 2, c.S_BLK, "xq2")
            if sb == 0 and hp == 2 and c.NSB > 1:
                qT_tiles[1] = qpool.tile([128, c.JC * c.S_BLK], DT,
                                         tag="qT", name="qT1")
                q1 = emit_qT_mms(1, xq_tiles[1], qT_tiles[1])
                fillers[idx] += q1[:16]
                fillers[min(idx + 1, len(units) - 1)] += q1[16:]
            if sb >= 1 and hp == 0 and sb + 1 < c.NSB:
                if sb + 2 < c.NSB and sb + 2 not in xq_tiles:
                    def pfq(sb=sb):
                        xq_tiles[sb + 2] = load_x(xqpool, xqT, sb + 2,
                                                  c.S_BLK, f"xq{sb + 2}")
                    fillers[idx].append(pfq)
                qT_tiles[sb + 1] = qpool.tile([128, c.JC * c.S_BLK], DT,
                                              tag="qT", name=f"qT{sb + 1}")
                qops = emit_qT_mms(sb + 1, xq_tiles[sb + 1], qT_tiles[sb + 1])
                for j in range(4):
                    fillers[min(idx + j, len(units) - 1)] += qops[j * 8:(j + 1) * 8]
            catT = cat_tiles[sb]
            qT = qT_tiles[sb]
            key = (sb, hp)
            pv_state = {}
            stage_runs[key] = (
                lambda cur_idx, sb=sb, hp=hp, catT=catT, pv_state=pv_state:
                emit_stage(sb, hp, catT, pv_state, cur_idx))
            flist = fillers[idx]
            fpos = 0
            for t in range(c.TCn):
                if sb == 0 and hp <= 1:
                    lag = PVLAG0
                elif idx == len(units) - 1:
                    lag = max(6, PVLAG - 2 * t)
                else:
                    lag = PVLAG
                kcol = slice(hp * c.S + t * 128, hp * c.S + (t + 1) * 128)
                qcol = slice(hp * c.S_BLK, (hp + 1) * c.S_BLK)
                # fillers FIRST: the PE queue is in-order, so anything queued
                # behind a sem-waiting scores matmul stalls with it. Feeding
                # the quota up front keeps the PE busy through the exp wait
                # (and keeps its p-state high).
                want = (len(flist) * (t + 1)) // c.TCn
                while fpos < want:
                    flist[fpos]()
                    fpos += 1
                # backstop: if pops are v-gated and the queue nears the
                # es-ring capacity, pull fillers forward (advancing the v'
                # copies) and retry pops after each one
                while len(pvq) >= 34 and fpos < len(flist):
                    flist[fpos]()
                    fpos += 1
                    pop_pv(len(pvq) - lag, idx)
                pop_pv(len(pvq) - lag, idx)
                # One 2-bank psum tile [A(512) | B(512)] per t-chunk: head A
                # rows 0:64, head B rows 64:128 (concurrent row-split pair);
                # ONE exp covers both heads.
                ps2 = pspool.tile([128, 2 * c.S_BLK], f32, tag="ps",
                                  name=f"ps2_{sb}_{hp}_{t}")
                nc.tensor.matmul(
                    ps2[:, 0:c.S_BLK],
                    kT_sb[0:64, kcol], qT[0:64, qcol],
                    start=True, stop=True)
                nc.tensor.matmul(
                    ps2[:, c.S_BLK:2 * c.S_BLK],
                    kT_sb[64:128, kcol], qT[64:128, qcol],
                    start=True, stop=True)
                es_t = epool.tile([128, 2 * c.S_BLK], EXPDT, tag="exp",
                                  name=f"es{sb}_{hp}_{t}")
                nc.scalar.activation(
                    es_t[:], ps2[:], mybir.ActivationFunctionType.Exp,
                    scale=SCALE)
                pvq.extend(make_pv_ops(key, sb, hp, t, es_t, pv_state))
            while fpos < len(flist):
                flist[fpos]()
                fpos += 1
        # ---- drain ----
        pop_pv(len(pvq), len(units) - 1)
        for op in pending_tail:
            op()

    nc.compile()
    return nc


def shard_inputs(inputs: dict, cfg: Cfg, DT=mybir.dt.bfloat16):
    """Full inputs -> list of 8 per-core in_maps (numpy)."""
    npdt = DT_NP[DT]
    q, k, v = inputs["queries"], inputs["keys"], inputs["values"]
    Wq, Wk, Wv = inputs["Wq"], inputs["Wk"], inputs["Wv"]
    Wout = inputs["Wout"]
    B = q.shape[0]
    maps = []
    WoutT = np.ascontiguousarray(Wout.T)  # [i, o]
    for core in range(2 * B):
        b, half = divmod(core, 2)
        hs = slice(half * cfg.HL, (half + 1) * cfg.HL)
        i0 = half * cfg.JW
        # wo: [JW, D] -> [HL heads, DK, D] -> [DK, HL, D] -> [DK, HL*D]
        wo = WoutT[i0:i0 + cfg.JW].reshape(cfg.HL, cfg.DK, cfg.D)
        wo = np.ascontiguousarray(wo.transpose(1, 0, 2).reshape(
            cfg.DK, cfg.HL * cfg.D))
        maps.append({
            "xqT": np.ascontiguousarray(q[b].T).astype(npdt),
            "xkT": np.ascontiguousarray(k[b].T).astype(npdt),
            "xvT": np.ascontiguousarray(v[b].T).astype(npdt),
            "wq": np.ascontiguousarray(
                Wq[hs].transpose(1, 0, 2).reshape(cfg.D, cfg.JW)).astype(npdt),
            "wk": np.ascontiguousarray(
                Wk[hs].transpose(1, 0, 2).reshape(cfg.D, cfg.JW)).astype(npdt),
            "wv": np.ascontiguousarray(
                Wv[hs].transpose(1, 0, 2).reshape(cfg.D, cfg.JW)).astype(npdt),
            "woutT": wo.astype(npdt),
        })
    return maps


def gather_outputs(results, inputs):
    bout = inputs["bout"]
    B = inputs["queries"].shape[0]
    outs = []
    for b in range(B):
        outs.append(results[2 * b]["out"] + results[2 * b + 1]["out"] + bout)
    return np.stack(outs).astype(np.float32)


def percore_reference(in_map: dict, cfg: Cfg):
    """Numpy reference of what one core should produce (fp32 math)."""
    c = cfg
    xq = in_map["xqT"].astype(np.float32).T   # [S, D]
    xk = in_map["xkT"].astype(np.float32).T
    xv = in_map["xvT"].astype(np.float32).T
    wq = in_map["wq"].astype(np.float32)      # [D, JW]
    wk = in_map["wk"].astype(np.float32)
    wv = in_map["wv"].astype(np.float32)
    wo = in_map["woutT"].astype(np.float32)   # [DK, HL*D]
    q = xq @ wq                               # [S, JW]
    k = xk @ wk
    v = xv @ wv
    out = np.zeros((c.S, c.D), dtype=np.float32)
    for h in range(c.HL):
        sl = slice(h * c.DK, (h + 1) * c.DK)
        s = (q[:, sl] @ k[:, sl].T) / np.sqrt(c.DK)
        e = np.exp(s)
        p = e / e.sum(axis=1, keepdims=True)
        wvh = p @ v[:, sl]                    # [S, DK]
        out += wvh @ wo[:, h * c.D:(h + 1) * c.D]
    return out

# ----------------------------------------------------------------------------
# Self-contained entry point: kernel(**inputs) -> full [B, S, D] output.
# ----------------------------------------------------------------------------
_NC_CACHE = {}


def _get_nc():
    key = "attn"
    if key not in _NC_CACHE:
        _NC_CACHE[key] = build_nc(Cfg(), mybir.dt.bfloat16, num_devices=8)
    return _NC_CACHE[key]


def kernel(**inputs):
    """Full (unsharded) inputs -> full [4, 2048, 1024] float32 output.

    Shards across the 8 NeuronCores as (batch x head-half), runs the Bass
    kernel SPMD, and gathers: out[b] = partial(core 2b) + partial(core 2b+1)
    + bias (row-sharded fc_out -> partial-sum reduction at gather time).
    """
    from concourse.bass_utils import run_bass_kernel_spmd

    inputs = {k: np.asarray(v) for k, v in inputs.items()}
    cfg = Cfg()
    nc = _get_nc()
    maps = shard_inputs(inputs, cfg, mybir.dt.bfloat16)
    res = run_bass_kernel_spmd(nc, maps, core_ids=list(range(8)), trace=False)
    return gather_outputs(res.results, inputs)


# revision 12
# speedup vs baseline: 1.0265x; 1.0265x over previous
"""Multi-head attention Bass/Tile kernel for TRN2, sharded 8 ways.

Sharding: core c handles batch b = c//2 and heads half = c%2 (8 of 16 heads).
Each core computes, for its batch and its 8 heads:
  q/k/v projections -> scoresT = K @ Q^T (per head, [t, s] layout) -> exp ->
  per-head PV: 16 accumulating K=128 matmuls into a [65, 512] psum tile,
  where lhsT = v' (64 value cols + a ones column) so psum row 64 ends up
  holding the softmax denominator sum_t exp for free -> reciprocal of that
  row (psum[64:65] -> sbuf[0:1], HW-verified) -> gpsimd partition_broadcast
  -> one DVE mul into catT -> partial output projection. Host sums the two
  partials per batch and adds the bias.

v2 vs the earlier quad-PV design: the DVE exp-sum chain (15 adds/unit), the
ones-matmul denominators and the psum t-half folds are all gone; both heads
of a pair keep all data on partitions 0:64 (no partition-shifting DVE ops,
which corrupt on HW). catT is [64, 8*512] (one 512-col block per head) and
the output projection contracts i in 8 chunks of K=64; Wout is reordered on
the host to match.

Layout choices (no transposes anywhere):
  xT     [D, S]  : host-pretransposed activations (d on partitions)
  wq/wk  [D, H*dk] : lhsT layout for qT/kT = W^T @ xT
  wv     [D, H*dk] : rhs layout for v = xT^T @ wv  ([t, vdim], natural)
  kT     [H*dk, S]: j on partitions -> head-pair p lives in 128-row chunk p
  scoresT[t, s]   : lhsT=kT [j,t] rows 0:64 / 64:128 (concurrent pair)
  v'     [t, h*65+0:64]=v, col h*65+64 = ones (memset preset)
  pv     [65, 512] psum: rows 0:64 = sum_t exp*v, row 64 = sum_t exp
  catT   [64, 8*S_BLK]: block 2*hp+head, rows = dv
  out    [s, o]   : lhsT=catT block [64, s-chunk] (K=64), rhs=wo block

HW pitfalls baked in (learned on-device):
  - no partition-shifting DVE copies; the only cross-partition moves are
    InstReciprocal psum[64:65]->sbuf[0:1] (verified) and gpsimd
    partition_broadcast
  - matmul free dim capped at 512; psum tiles are bank-aligned
"""

from contextlib import ExitStack
from dataclasses import dataclass

import numpy as np
import ml_dtypes

import concourse.bass as bass  # noqa: F401
import concourse.tile as tile
from concourse import bacc, mybir


@dataclass
class Cfg:
    D: int = 1024      # model dim
    S: int = 2048      # sequence length (queries == keys)
    HL: int = 8        # heads per core
    DK: int = 64       # head dim
    S_BLK: int = 512   # query block (matmul free dim)
    T_BLK: int = 512   # t block in projection phase

    @property
    def DC(self):
        return self.D // 128

    @property
    def NSB(self):
        return self.S // self.S_BLK

    @property
    def TBn(self):
        return self.S // self.T_BLK

    @property
    def TCn(self):
        return self.S // 128

    @property
    def JW(self):
        return self.HL * self.DK

    @property
    def JC(self):
        return self.JW // 128

    @property
    def VW(self):
        return self.DK + 1

    @property
    def OB(self):
        return min(512, self.D)


DT_NP = {
    mybir.dt.bfloat16: ml_dtypes.bfloat16,
    mybir.dt.float32: np.float32,
    mybir.dt.float32r: np.float32,
}


def build_nc(cfg: Cfg, DT=mybir.dt.bfloat16, num_devices: int = 8):
    c = cfg
    f32 = mybir.dt.float32
    EXPDT = DT if DT == mybir.dt.bfloat16 else f32
    SCALE = 1.0 / float(np.sqrt(c.DK))
    nc = bacc.Bacc("TRN2", target_bir_lowering=False, debug=False,
                   num_devices=num_devices)

    xqT = nc.dram_tensor("xqT", [c.D, c.S], DT, kind="ExternalInput").ap()
    xkT = nc.dram_tensor("xkT", [c.D, c.S], DT, kind="ExternalInput").ap()
    xvT = nc.dram_tensor("xvT", [c.D, c.S], DT, kind="ExternalInput").ap()
    wq_d = nc.dram_tensor("wq", [c.D, c.JW], DT, kind="ExternalInput").ap()
    wk_d = nc.dram_tensor("wk", [c.D, c.JW], DT, kind="ExternalInput").ap()
    wv_d = nc.dram_tensor("wv", [c.D, c.JW], DT, kind="ExternalInput").ap()
    wo_d = nc.dram_tensor("woutT", [c.JW, c.D], DT, kind="ExternalInput").ap()
    out_d = nc.dram_tensor("out", [c.S, c.D], f32, kind="ExternalOutput").ap()

    from collections import deque

    with tile.TileContext(nc) as tc, ExitStack() as es:
        wpool = es.enter_context(tc.tile_pool(name="weights", bufs=1))
        kvpool = es.enter_context(tc.tile_pool(name="kv", bufs=1))
        xkpool = es.enter_context(tc.tile_pool(name="xk", bufs=4))
        xqpool = es.enter_context(tc.tile_pool(name="xq", bufs=2))
        xvpool = es.enter_context(tc.tile_pool(name="xv", bufs=2))
        qpool = es.enter_context(tc.tile_pool(name="q", bufs=2))
        epool = es.enter_context(tc.tile_pool(name="exp", bufs=18))
        cpool = es.enter_context(tc.tile_pool(name="cat", bufs=2))
        opool = es.enter_context(tc.tile_pool(name="o", bufs=2))
        rpool = es.enter_context(tc.tile_pool(name="r", bufs=1))
        pspool = es.enter_context(tc.tile_pool(name="ps", bufs=2, space="PSUM"))
        pvpool = es.enter_context(tc.tile_pool(name="pv", bufs=2, space="PSUM"))
        fppool = es.enter_context(tc.tile_pool(name="fp", bufs=2, space="PSUM"))

        def load_w_dmaj(dram, width, tag):
            t = wpool.tile([128, c.DC * width], DT, tag=tag, name=tag)
            for d in range(c.DC):
                eng = nc.sync if d % 2 == 0 else nc.gpsimd
                eng.dma_start(t[:, d * width:(d + 1) * width],
                              dram[d * 128:(d + 1) * 128, :])
            return t

        def load_x(pool, dram, blk, width, name):
            t = pool.tile([128, c.DC * width], DT, tag="x", name=name)
            for d in range(c.DC):
                eng = nc.sync if d % 2 == 0 else nc.gpsimd
                eng.dma_start(
                    t[:, d * width:(d + 1) * width],
                    dram[d * 128:(d + 1) * 128, blk * width:(blk + 1) * width])
            return t

        NT = c.T_BLK

        # ---- head DMAs: wk + xk0 lead (kT tb0/jc0 inline), then wq/xq0 for
        # qT(0) jc0; the rest arrives while the pipeline runs.
        wk_sb = load_w_dmaj(wk_d, c.JW, "wk")
        kT_sb = kvpool.tile([128, c.JC * c.S], DT, tag="kT", name="kT")
        xk_tiles = [load_x(xkpool, xkT, 0, NT, "xk0")]
        wq_sb = load_w_dmaj(wq_d, c.JW, "wq")
        xq_tiles = {0: load_x(xqpool, xqT, 0, c.S_BLK, "xq0")}
        xk_tiles += [load_x(xkpool, xkT, tb, NT, f"xk{tb}")
                     for tb in range(1, c.TBn)]
        wv_sb = load_w_dmaj(wv_d, c.JW, "wv")
        v_sb = kvpool.tile([128, c.TCn * c.HL * c.VW], DT, tag="v", name="v")
        nc.gpsimd.memset(v_sb[:], 1.0)  # ones columns preset
        xv_tiles = {0: load_x(xvpool, xvT, 0, NT, "xv0"),
                    1: load_x(xvpool, xvT, 1, NT, "xv1")}
        xq_tiles[1] = load_x(xqpool, xqT, 1, c.S_BLK, "xq1")
        wo_sb = wpool.tile([128, c.JC * c.D], DT, tag="wo", name="wo")
        for ic in range(c.JC):
            eng = nc.sync if ic % 2 == 0 else nc.gpsimd
            eng.dma_start(wo_sb[:, ic * c.D:(ic + 1) * c.D],
                          wo_d[ic * 128:(ic + 1) * 128, :])

        def kT_ops(tb, jc):
            """8 MM closures computing kT chunk jc for t-block tb."""
            box = {}

            def mk(d):
                def op():
                    if d == 0:
                        box["ps"] = fppool.tile([128, NT], f32, tag="fp",
                                                name=f"psk{tb}_{jc}")
                    # col-split pair: both halves co-run in the PE array
                    for h in range(2):
                        nc.tensor.matmul(
                            box["ps"][h * 64:(h + 1) * 64, :],
                            wk_sb[:, d * c.JW + jc * 128 + h * 64:
                                  d * c.JW + jc * 128 + (h + 1) * 64],
                            xk_tiles[tb][:, d * NT:(d + 1) * NT],
                            start=(d == 0), stop=(d == c.DC - 1),
                            skip_group_check=True)
                    if d == c.DC - 1:
                        nc.vector.tensor_copy(
                            kT_sb[:, jc * c.S + tb * NT: jc * c.S + (tb + 1) * NT],
                            box["ps"][:])
                return op
            return [mk(d) for d in range(c.DC)]

        # count of v' chunk-groups whose SBUF copy has been EMITTED — PV pops
        # for sb-0 units must not overtake this.
        v_done = [0]

        def v_ops(tb):
            """v' projection closures for t-block tb (+ trailing xv prefetch)."""
            ops = []
            for tt in range(NT // 128):
                g = tb * (NT // 128) + tt
                box = {}

                def mk(d, g=g, tt=tt, tb=tb, box=box):
                    def op():
                        if d == 0:
                            box["ps"] = fppool.tile([128, c.JW], f32, tag="fp",
                                                    name=f"psv{g}")
                        for h in range(2):
                            nc.tensor.matmul(
                                box["ps"][h * 64:(h + 1) * 64, :],
                                xv_tiles[tb][:, d * NT + tt * 128 + h * 64:
                                             d * NT + tt * 128 + (h + 1) * 64],
                                wv_sb[:, d * c.JW:(d + 1) * c.JW],
                                start=(d == 0),
                                stop=(d == c.DC - 1),
                                skip_group_check=True)
                        if d == c.DC - 1:
                            dst = v_sb[:, g * c.HL * c.VW:(g + 1) * c.HL * c.VW]
                            dst3 = dst.rearrange("p (h w) -> p h w",
                                                 w=c.VW)[:, :, 0:c.DK]
                            src3 = box["ps"][:].rearrange("p (h w) -> p h w",
                                                          w=c.DK)
                            nc.vector.tensor_copy(dst3, src3)
                            v_done[0] = g + 1
                    return op
                ops += [mk(d) for d in range(c.DC)]
            if tb + 2 < c.TBn:
                def pf(tb=tb):
                    xv_tiles[tb + 2] = load_x(xvpool, xvT, tb + 2, NT,
                                              f"xv{tb + 2}")
                ops.append(pf)
            return ops

        def emit_qT_mms(sb, xq, qT):
            """32 MM closures (jc-major); last per jc copies psum -> qT chunk
            jc (head A rows 0:64, head B rows 64:128 — natural layout)."""
            ops = []
            psq_box = {}

            def mk(jc, d):
                def op():
                    if d == 0:
                        psq_box[jc] = fppool.tile([128, c.S_BLK], f32, tag="fp",
                                                  name=f"psq{sb}_{jc}")
                    for h in range(2):
                        nc.tensor.matmul(
                            psq_box[jc][h * 64:(h + 1) * 64, :],
                            wq_sb[:, d * c.JW + jc * 128 + h * 64:
                                  d * c.JW + jc * 128 + (h + 1) * 64],
                            xq[:, d * c.S_BLK:(d + 1) * c.S_BLK],
                            start=(d == 0), stop=(d == c.DC - 1),
                            skip_group_check=True)
                    if d == c.DC - 1:
                        nc.vector.tensor_copy(
                            qT[:, jc * c.S_BLK:(jc + 1) * c.S_BLK],
                            psq_box[jc][:])
                return op
            for jc in range(c.JC):
                for d in range(c.DC):
                    ops.append(mk(jc, d))
            return ops

        def emit_outproj_mms(sb, catT):
            """Closures: per (sc, oc): 4 K=128 ic-MMs (each a co-running
            col-split half-pair) into a 1-bank psum, then copy + DMA out."""
            ops = []
            po_box = {}
            NIC = c.JC

            def mk(sc, oc, ic):
                def op():
                    if ic == 0:
                        po_box[(sc, oc)] = fppool.tile(
                            [128, c.OB], f32, tag="fp", name=f"po{sb}_{sc}_{oc}")
                    po = po_box[(sc, oc)]
                    for h in range(2):
                        nc.tensor.matmul(
                            po[h * 64:(h + 1) * 64, :],
                            catT[:, ic * c.S_BLK + sc * 128 + h * 64:
                                 ic * c.S_BLK + sc * 128 + (h + 1) * 64],
                            wo_sb[:, ic * c.D + oc * c.OB:
                                  ic * c.D + (oc + 1) * c.OB],
                            start=(ic == 0), stop=(ic == NIC - 1),
                            skip_group_check=True)
                    if ic == NIC - 1:
                        ot = opool.tile([128, c.OB], f32, tag="ot",
                                        name=f"ot{sb}_{sc}_{oc}")
                        nc.vector.tensor_copy(ot[:], po[:])
                        eng = nc.sync if (sc + oc) % 2 == 0 else nc.gpsimd
                        eng.dma_start(
                            out_d[sb * c.S_BLK + sc * 128:
                                  sb * c.S_BLK + (sc + 1) * 128,
                                  oc * c.OB:(oc + 1) * c.OB],
                            ot[:])
                return op
            for sc in range(c.S_BLK // 128):
                for oc in range(c.D // c.OB):
                    for ic in range(NIC):
                        ops.append(mk(sc, oc, ic))
            return ops

        # ---- slim prologue: only what unit (0,0) chunk 0 needs ----
        for op in kT_ops(0, 0):
            op()
        qT_tiles = {0: qpool.tile([128, c.JC * c.S_BLK], DT, tag="qT",
                                  name="qT0")}
        q0 = emit_qT_mms(0, xq_tiles[0], qT_tiles[0])
        for op in q0[0:8]:
            op()

        # ---- units + filler lists ----
        units = [(sb, hp) for sb in range(c.NSB) for hp in range(c.JC)]
        fillers = [[] for _ in units]
        # deadlines: kT(tb,jc) by chunk 4*tb of unit (0,jc); q0 jc by end of
        # unit (0,jc-1); v groups gated by v_done + deep PV lag.
        fillers[0] += kT_ops(1, 0) + kT_ops(2, 0) + kT_ops(3, 0) \
            + v_ops(0) + v_ops(1) + q0[8:16] + kT_ops(0, 1)
        fillers[1] += kT_ops(1, 1) + kT_ops(2, 1) + kT_ops(3, 1) \
            + v_ops(2) + v_ops(3) + q0[16:24] + kT_ops(0, 2)
        fillers[2] += kT_ops(1, 2) + kT_ops(2, 2) + kT_ops(3, 2) \
            + q0[24:32] + kT_ops(0, 3)
        fillers[3] += kT_ops(1, 3) + kT_ops(2, 3) + kT_ops(3, 3)

        cat_tiles = {}

        # ---- lagged-PV queue machinery (2 ops per t-chunk now) ----
        pvq = deque()        # (key, op, islast, need_v)
        stage_runs = {}      # key -> closure(cur_idx)
        pending_tail = []    # ops deferred past the last unit (drain)
        PVLAG = 64           # one full unit behind (4 ops/chunk)
        PVLAG0 = 64          # deep hold while sb-0's v' fillers land

        def emit_stage(sb, hp, catT, pv_state, cur_idx):
            """Normalize both heads from their pv psum tiles: reciprocal of
            the ones-row (psum[64:65]->sbuf[0:1], verified on HW), broadcast,
            one mul per head into catT block 2*hp+head. All ops stay on
            partitions 0:64 — no partition shifts."""
            X, Y = pv_state["X"], pv_state["Y"]
            for h in range(2):
                rti = rpool.tile([1, c.S_BLK], f32, tag=f"rti{h}",
                                 name=f"rti{h}_{sb}_{hp}")
                nc.vector.reciprocal(rti[:], Y[h * 64:h * 64 + 1, :])
                rb = rpool.tile([128, c.S_BLK], f32, tag=f"rb{h}",
                                name=f"rb{h}_{sb}_{hp}")
                nc.gpsimd.partition_broadcast(rb[:], rti[:])
                nc.vector.tensor_mul(
                    catT[h * 64:(h + 1) * 64,
                         hp * c.S_BLK:(hp + 1) * c.S_BLK],
                    X[h * 64:(h + 1) * 64, :], rb[h * 64:(h + 1) * 64, :])
            if hp == c.JC - 1:
                oops = emit_outproj_mms(sb, catT)
                splits = [(0, 12), (12, 24), (24, 32)]
                for j, (lo, hi) in enumerate(splits):
                    tgt = cur_idx + 1 + j
                    if tgt < len(units):
                        fillers[tgt] += oops[lo:hi]
                    else:
                        pending_tail.extend(oops[lo:hi])

        def make_pv_ops(key, sb, hp, t, es_tile, pv_state):
            """4 accumulating matmuls per t-chunk, in two co-running pairs:
            - value pair: head A -> X[0:64] (col group 0), head B ->
              X[64:128] (col group 64); K=128, same bank, concurrent.
            - denominator pair: ones-column lhsT, M=1: head A -> Y[0:1],
              head B -> Y[64:65]; concurrent col strips.
            Only the first matmul touching each bank carries start=True
            (start clears the whole bank)."""
            W = c.HL * c.VW
            first = (t == 0)
            last = (t == c.TCn - 1)

            def mkv(h):
                vsl = slice(t * W + (2 * hp + h) * c.VW,
                            t * W + (2 * hp + h) * c.VW + c.DK)
                ecol = slice(h * c.S_BLK, (h + 1) * c.S_BLK)

                def op():
                    if first and h == 0:
                        pv_state["X"] = pvpool.tile(
                            [128, c.S_BLK], f32, tag="pv",
                            name=f"pvX_{sb}_{hp}")
                    nc.tensor.matmul(
                        pv_state["X"][h * 64:(h + 1) * 64, :],
                        v_sb[:, vsl], es_tile[:, ecol],
                        start=first, stop=last,
                        skip_group_check=True)
                return op

            def mkd(h):
                osl = slice(t * W + (2 * hp + h) * c.VW + c.DK,
                            t * W + (2 * hp + h) * c.VW + c.VW)
                ecol = slice(h * c.S_BLK, (h + 1) * c.S_BLK)

                def op():
                    if first and h == 0:
                        pv_state["Y"] = pvpool.tile(
                            [128, c.S_BLK], f32, tag="pv",
                            name=f"pvY_{sb}_{hp}")
                    nc.tensor.matmul(
                        pv_state["Y"][h * 64:h * 64 + 1, :],
                        v_sb[:, osl], es_tile[:, ecol],
                        start=first, stop=last,
                        skip_group_check=True)
                return op
            need_v = t + 1 if sb == 0 else 0
            return [(key, mkv(0), False, need_v),
                    (key, mkv(1), False, need_v),
                    (key, mkd(0), False, need_v),
                    (key, mkd(1), last, need_v)]

        def pop_pv(n, cur_idx):
            for _ in range(n):
                if not pvq:
                    return
                if pvq[0][3] > v_done[0]:
                    return  # its v' chunk copy not yet emitted
                key, op, islast, _ = pvq.popleft()
                op()
                if islast:
                    stage_runs.pop(key)(cur_idx)

        # ---- main pipeline over units ----
        for idx, (sb, hp) in enumerate(units):
            if hp == 0:
                cat_tiles[sb] = cpool.tile([128, c.JC * c.S_BLK], DT,
                                           tag="cat", name=f"catT{sb}")
            if sb == 0 and hp == 3 and c.NSB > 2:
                xq_tiles[2] = load_x(xqpool, xqT, 2, c.S_BLK, "xq2")
            if sb == 0 and hp == 2 and c.NSB > 1:
                qT_tiles[1] = qpool.tile([128, c.JC * c.S_BLK], DT,
                                         tag="qT", name="qT1")
                q1 = emit_qT_mms(1, xq_tiles[1], qT_tiles[1])
                fillers[idx] += q1[:16]
                fillers[min(idx + 1, len(units) - 1)] += q1[16:]
            if sb >= 1 and hp == 0 and sb + 1 < c.NSB:
                if sb + 2 < c.NSB and sb + 2 not in xq_tiles:
                    def pfq(sb=sb):
                        xq_tiles[sb + 2] = load_x(xqpool, xqT, sb + 2,
                                                  c.S_BLK, f"xq{sb + 2}")
                    fillers[idx].append(pfq)
                qT_tiles[sb + 1] = qpool.tile([128, c.JC * c.S_BLK], DT,
                                              tag="qT", name=f"qT{sb + 1}")
                qops = emit_qT_mms(sb + 1, xq_tiles[sb + 1], qT_tiles[sb + 1])
                for j in range(4):
                    fillers[min(idx + j, len(units) - 1)] += qops[j * 8:(j + 1) * 8]
            catT = cat_tiles[sb]
            qT = qT_tiles[sb]
            key = (sb, hp)
            pv_state = {}
            stage_runs[key] = (
                lambda cur_idx, sb=sb, hp=hp, catT=catT, pv_state=pv_state:
                emit_stage(sb, hp, catT, pv_state, cur_idx))
            flist = fillers[idx]
            fpos = 0
            for t in range(c.TCn):
                if sb == 0 and hp <= 1:
                    lag = PVLAG0
                elif idx == len(units) - 1:
                    lag = max(12, PVLAG - 4 * t)
                else:
                    lag = PVLAG
                kcol = slice(hp * c.S + t * 128, hp * c.S + (t + 1) * 128)
                qcol = slice(hp * c.S_BLK, (hp + 1) * c.S_BLK)
                # fillers FIRST: the PE queue is in-order, so anything queued
                # behind a sem-waiting scores matmul stalls with it. Feeding
                # the quota up front keeps the PE busy through the exp wait
                # (and keeps its p-state high).
                want = (len(flist) * (t + 1)) // c.TCn
                while fpos < want:
                    flist[fpos]()
                    fpos += 1
                # backstop: if pops are v-gated and the queue nears the
                # es-ring capacity, pull fillers forward (advancing the v'
                # copies) and retry pops after each one
                while len(pvq) >= 72 and fpos < len(flist):
                    flist[fpos]()
                    fpos += 1
                    pop_pv(len(pvq) - lag, idx)
                pop_pv(len(pvq) - lag, idx)
                # One 2-bank psum tile [A(512) | B(512)] per t-chunk: head A
                # rows 0:64, head B rows 64:128 (concurrent row-split pair);
                # ONE exp covers both heads.
                ps2 = pspool.tile([128, 2 * c.S_BLK], f32, tag="ps",
                                  name=f"ps2_{sb}_{hp}_{t}")
                nc.tensor.matmul(
                    ps2[:, 0:c.S_BLK],
                    kT_sb[0:64, kcol], qT[0:64, qcol],
                    start=True, stop=True)
                nc.tensor.matmul(
                    ps2[:, c.S_BLK:2 * c.S_BLK],
                    kT_sb[64:128, kcol], qT[64:128, qcol],
                    start=True, stop=True)
                es_t = epool.tile([128, 2 * c.S_BLK], EXPDT, tag="exp",
                                  name=f"es{sb}_{hp}_{t}")
                nc.scalar.activation(
                    es_t[:], ps2[:], mybir.ActivationFunctionType.Exp,
                    scale=SCALE)
                pvq.extend(make_pv_ops(key, sb, hp, t, es_t, pv_state))
            while fpos < len(flist):
                flist[fpos]()
                fpos += 1
        # ---- drain ----
        pop_pv(len(pvq), len(units) - 1)
        for op in pending_tail:
            op()

    nc.compile()
    return nc


def shard_inputs(inputs: dict, cfg: Cfg, DT=mybir.dt.bfloat16):
    """Full inputs -> list of 8 per-core in_maps (numpy)."""
    npdt = DT_NP[DT]
    q, k, v = inputs["queries"], inputs["keys"], inputs["values"]
    Wq, Wk, Wv = inputs["Wq"], inputs["Wk"], inputs["Wv"]
    Wout = inputs["Wout"]
    B = q.shape[0]
    maps = []
    WoutT = np.ascontiguousarray(Wout.T)  # [i, o]
    for core in range(2 * B):
        b, half = divmod(core, 2)
        hs = slice(half * cfg.HL, (half + 1) * cfg.HL)
        i0 = half * cfg.JW
        maps.append({
            "xqT": np.ascontiguousarray(q[b].T).astype(npdt),
            "xkT": np.ascontiguousarray(k[b].T).astype(npdt),
            "xvT": np.ascontiguousarray(v[b].T).astype(npdt),
            "wq": np.ascontiguousarray(
                Wq[hs].transpose(1, 0, 2).reshape(cfg.D, cfg.JW)).astype(npdt),
            "wk": np.ascontiguousarray(
                Wk[hs].transpose(1, 0, 2).reshape(cfg.D, cfg.JW)).astype(npdt),
            "wv": np.ascontiguousarray(
                Wv[hs].transpose(1, 0, 2).reshape(cfg.D, cfg.JW)).astype(npdt),
            "woutT": np.ascontiguousarray(WoutT[i0:i0 + cfg.JW]).astype(npdt),
        })
    return maps


def gather_outputs(results, inputs):
    bout = inputs["bout"]
    B = inputs["queries"].shape[0]
    outs = []
    for b in range(B):
        outs.append(results[2 * b]["out"] + results[2 * b + 1]["out"] + bout)
    return np.stack(outs).astype(np.float32)


def percore_reference(in_map: dict, cfg: Cfg):
    """Numpy reference of what one core should produce (fp32 math)."""
    c = cfg
    xq = in_map["xqT"].astype(np.float32).T   # [S, D]
    xk = in_map["xkT"].astype(np.float32).T
    xv = in_map["xvT"].astype(np.float32).T
    wq = in_map["wq"].astype(np.float32)      # [D, JW]
    wk = in_map["wk"].astype(np.float32)
    wv = in_map["wv"].astype(np.float32)
    wo = in_map["woutT"].astype(np.float32)   # [JW, D]
    q = xq @ wq                               # [S, JW]
    k = xk @ wk
    v = xv @ wv
    cat = np.zeros((c.S, c.JW), dtype=np.float32)
    for h in range(c.HL):
        sl = slice(h * c.DK, (h + 1) * c.DK)
        s = (q[:, sl] @ k[:, sl].T) / np.sqrt(c.DK)
        e = np.exp(s)
        p = e / e.sum(axis=1, keepdims=True)
        cat[:, sl] = p @ v[:, sl]
    return cat @ wo

# ----------------------------------------------------------------------------
# Self-contained entry point: kernel(**inputs) -> full [B, S, D] output.
# ----------------------------------------------------------------------------
_NC_CACHE = {}


def _get_nc():
    key = "attn"
    if key not in _NC_CACHE:
        _NC_CACHE[key] = build_nc(Cfg(), mybir.dt.bfloat16, num_devices=8)
    return _NC_CACHE[key]


def kernel(**inputs):
    """Full (unsharded) inputs -> full [4, 2048, 1024] float32 output.

    Shards across the 8 NeuronCores as (batch x head-half), runs the Bass
    kernel SPMD, and gathers: out[b] = partial(core 2b) + partial(core 2b+1)
    + bias (row-sharded fc_out -> partial-sum reduction at gather time).
    """
    from concourse.bass_utils import run_bass_kernel_spmd

    inputs = {k: np.asarray(v) for k, v in inputs.items()}
    cfg = Cfg()
    nc = _get_nc()
    maps = shard_inputs(inputs, cfg, mybir.dt.bfloat16)
    res = run_bass_kernel_spmd(nc, maps, core_ids=list(range(8)), trace=False)
    return gather_outputs(res.results, inputs)


# revision 13
# speedup vs baseline: 1.2288x; 1.1971x over previous
"""Multi-head attention Bass/Tile kernel for TRN2, sharded 8 ways.

Sharding: core c handles batch b = c//2 and heads half = c%2 (8 of 16 heads).
Each core computes, for its batch and its 8 heads:
  q/k/v projections -> scoresT = K @ Q^T (per head, [t, s] layout) -> exp ->
  per-head PV: 16 accumulating K=128 matmuls into a [65, 512] psum tile,
  where lhsT = v' (64 value cols + a ones column) so psum row 64 ends up
  holding the softmax denominator sum_t exp for free -> reciprocal of that
  row (psum[64:65] -> sbuf[0:1], HW-verified) -> gpsimd partition_broadcast
  -> one DVE mul into catT -> partial output projection. Host sums the two
  partials per batch and adds the bias.

v2 vs the earlier quad-PV design: the DVE exp-sum chain (15 adds/unit), the
ones-matmul denominators and the psum t-half folds are all gone; both heads
of a pair keep all data on partitions 0:64 (no partition-shifting DVE ops,
which corrupt on HW). catT is [64, 8*512] (one 512-col block per head) and
the output projection contracts i in 8 chunks of K=64; Wout is reordered on
the host to match.

Layout choices (no transposes anywhere):
  xT     [D, S]  : host-pretransposed activations (d on partitions)
  wq/wk  [D, H*dk] : lhsT layout for qT/kT = W^T @ xT
  wv     [D, H*dk] : rhs layout for v = xT^T @ wv  ([t, vdim], natural)
  kT     [H*dk, S]: j on partitions -> head-pair p lives in 128-row chunk p
  scoresT[t, s]   : lhsT=kT [j,t] rows 0:64 / 64:128 (concurrent pair)
  v'     [t, h*65+0:64]=v, col h*65+64 = ones (memset preset)
  pv     [65, 512] psum: rows 0:64 = sum_t exp*v, row 64 = sum_t exp
  catT   [64, 8*S_BLK]: block 2*hp+head, rows = dv
  out    [s, o]   : lhsT=catT block [64, s-chunk] (K=64), rhs=wo block

HW pitfalls baked in (learned on-device):
  - no partition-shifting DVE copies; the only cross-partition moves are
    InstReciprocal psum[64:65]->sbuf[0:1] (verified) and gpsimd
    partition_broadcast
  - matmul free dim capped at 512; psum tiles are bank-aligned
"""

from contextlib import ExitStack
from dataclasses import dataclass

import numpy as np
import ml_dtypes

import concourse.bass as bass  # noqa: F401
import concourse.tile as tile
from concourse import bacc, mybir


@dataclass
class Cfg:
    D: int = 1024      # model dim
    S: int = 2048      # sequence length (queries == keys)
    HL: int = 8        # heads per core
    DK: int = 64       # head dim
    S_BLK: int = 512   # query block (matmul free dim)
    T_BLK: int = 512   # t block in projection phase

    @property
    def DC(self):
        return self.D // 128

    @property
    def NSB(self):
        return self.S // self.S_BLK

    @property
    def TBn(self):
        return self.S // self.T_BLK

    @property
    def TCn(self):
        return self.S // 128

    @property
    def JW(self):
        return self.HL * self.DK

    @property
    def JC(self):
        return self.JW // 128

    @property
    def VW(self):
        return self.DK + 1

    @property
    def OB(self):
        return min(512, self.D)


DT_NP = {
    mybir.dt.bfloat16: ml_dtypes.bfloat16,
    mybir.dt.float32: np.float32,
    mybir.dt.float32r: np.float32,
}


def build_nc(cfg: Cfg, DT=mybir.dt.bfloat16, num_devices: int = 8):
    c = cfg
    f32 = mybir.dt.float32
    EXPDT = DT if DT == mybir.dt.bfloat16 else f32
    SCALE = 1.0 / float(np.sqrt(c.DK))
    nc = bacc.Bacc("TRN2", target_bir_lowering=False, debug=False,
                   num_devices=num_devices)

    xqT = nc.dram_tensor("xqT", [c.D, c.S], DT, kind="ExternalInput").ap()
    xkT = nc.dram_tensor("xkT", [c.D, c.S], DT, kind="ExternalInput").ap()
    xvT = nc.dram_tensor("xvT", [c.D, c.S], DT, kind="ExternalInput").ap()
    wq_d = nc.dram_tensor("wq", [c.D, c.JW], DT, kind="ExternalInput").ap()
    wk_d = nc.dram_tensor("wk", [c.D, c.JW], DT, kind="ExternalInput").ap()
    wv_d = nc.dram_tensor("wv", [c.D, c.JW], DT, kind="ExternalInput").ap()
    wo_d = nc.dram_tensor("woutT", [c.JW, c.D], DT, kind="ExternalInput").ap()
    out_d = nc.dram_tensor("out", [c.S, c.D], f32, kind="ExternalOutput").ap()

    from collections import deque

    with tile.TileContext(nc) as tc, ExitStack() as es:
        wpool = es.enter_context(tc.tile_pool(name="weights", bufs=1))
        kvpool = es.enter_context(tc.tile_pool(name="kv", bufs=1))
        xkpool = es.enter_context(tc.tile_pool(name="xk", bufs=4))
        xqpool = es.enter_context(tc.tile_pool(name="xq", bufs=2))
        xvpool = es.enter_context(tc.tile_pool(name="xv", bufs=2))
        qpool = es.enter_context(tc.tile_pool(name="q", bufs=2))
        epool = es.enter_context(tc.tile_pool(name="exp", bufs=18))
        cpool = es.enter_context(tc.tile_pool(name="cat", bufs=2))
        opool = es.enter_context(tc.tile_pool(name="o", bufs=2))
        rpool = es.enter_context(tc.tile_pool(name="r", bufs=1))
        pspool = es.enter_context(tc.tile_pool(name="ps", bufs=2, space="PSUM"))
        pvpool = es.enter_context(tc.tile_pool(name="pv", bufs=2, space="PSUM"))
        fppool = es.enter_context(tc.tile_pool(name="fp", bufs=2, space="PSUM"))

        def load_w_dmaj(dram, width, tag):
            t = wpool.tile([128, c.DC * width], DT, tag=tag, name=tag)
            for d in range(c.DC):
                eng = nc.sync if d % 2 == 0 else nc.gpsimd
                eng.dma_start(t[:, d * width:(d + 1) * width],
                              dram[d * 128:(d + 1) * 128, :])
            return t

        def load_x(pool, dram, blk, width, name):
            t = pool.tile([128, c.DC * width], DT, tag="x", name=name)
            for d in range(c.DC):
                eng = nc.sync if d % 2 == 0 else nc.gpsimd
                eng.dma_start(
                    t[:, d * width:(d + 1) * width],
                    dram[d * 128:(d + 1) * 128, blk * width:(blk + 1) * width])
            return t

        NT = c.T_BLK

        # ---- head DMAs: wk + xk0 lead (kT tb0/jc0 inline), then wq/xq0 for
        # qT(0) jc0; the rest arrives while the pipeline runs.
        wk_sb = load_w_dmaj(wk_d, c.JW, "wk")
        kT_sb = kvpool.tile([128, c.JC * c.S], DT, tag="kT", name="kT")
        xk_tiles = [load_x(xkpool, xkT, 0, NT, "xk0")]
        wq_sb = load_w_dmaj(wq_d, c.JW, "wq")
        xq_tiles = {0: load_x(xqpool, xqT, 0, c.S_BLK, "xq0")}
        xk_tiles += [load_x(xkpool, xkT, tb, NT, f"xk{tb}")
                     for tb in range(1, c.TBn)]
        wv_sb = load_w_dmaj(wv_d, c.JW, "wv")
        v_sb = kvpool.tile([128, c.TCn * c.HL * c.VW], DT, tag="v", name="v")
        nc.gpsimd.memset(v_sb[:], 1.0)  # ones columns preset
        xv_tiles = {0: load_x(xvpool, xvT, 0, NT, "xv0"),
                    1: load_x(xvpool, xvT, 1, NT, "xv1")}
        xq_tiles[1] = load_x(xqpool, xqT, 1, c.S_BLK, "xq1")
        wo_sb = wpool.tile([128, c.JC * c.D], DT, tag="wo", name="wo")
        for ic in range(c.JC):
            eng = nc.sync if ic % 2 == 0 else nc.gpsimd
            eng.dma_start(wo_sb[:, ic * c.D:(ic + 1) * c.D],
                          wo_d[ic * 128:(ic + 1) * 128, :])

        def kT_ops(tb, jc):
            """8 MM closures computing kT chunk jc for t-block tb."""
            box = {}

            def mk(d):
                def op():
                    if d == 0:
                        box["ps"] = fppool.tile([128, NT], f32, tag="fp",
                                                name=f"psk{tb}_{jc}")
                    # col-split pair: both halves co-run in the PE array
                    for h in range(2):
                        nc.tensor.matmul(
                            box["ps"][h * 64:(h + 1) * 64, :],
                            wk_sb[:, d * c.JW + jc * 128 + h * 64:
                                  d * c.JW + jc * 128 + (h + 1) * 64],
                            xk_tiles[tb][:, d * NT:(d + 1) * NT],
                            start=(d == 0), stop=(d == c.DC - 1),
                            skip_group_check=True)
                    if d == c.DC - 1:
                        nc.vector.tensor_copy(
                            kT_sb[:, jc * c.S + tb * NT: jc * c.S + (tb + 1) * NT],
                            box["ps"][:])
                return op
            return [mk(d) for d in range(c.DC)]

        # count of v' chunk-groups whose SBUF copy has been EMITTED — PV pops
        # for sb-0 units must not overtake this.
        v_done = [0]

        def v_ops(tb):
            """v' projection closures for t-block tb (+ trailing xv prefetch)."""
            ops = []
            for tt in range(NT // 128):
                g = tb * (NT // 128) + tt
                box = {}

                def mk(d, g=g, tt=tt, tb=tb, box=box):
                    def op():
                        if d == 0:
                            box["ps"] = fppool.tile([128, c.JW], f32, tag="fp",
                                                    name=f"psv{g}")
                        for h in range(2):
                            nc.tensor.matmul(
                                box["ps"][h * 64:(h + 1) * 64, :],
                                xv_tiles[tb][:, d * NT + tt * 128 + h * 64:
                                             d * NT + tt * 128 + (h + 1) * 64],
                                wv_sb[:, d * c.JW:(d + 1) * c.JW],
                                start=(d == 0),
                                stop=(d == c.DC - 1),
                                skip_group_check=True)
                        if d == c.DC - 1:
                            dst = v_sb[:, g * c.HL * c.VW:(g + 1) * c.HL * c.VW]
                            dst3 = dst.rearrange("p (h w) -> p h w",
                                                 w=c.VW)[:, :, 0:c.DK]
                            src3 = box["ps"][:].rearrange("p (h w) -> p h w",
                                                          w=c.DK)
                            nc.vector.tensor_copy(dst3, src3)
                            v_done[0] = g + 1
                    return op
                ops += [mk(d) for d in range(c.DC)]
            if tb + 2 < c.TBn:
                def pf(tb=tb):
                    xv_tiles[tb + 2] = load_x(xvpool, xvT, tb + 2, NT,
                                              f"xv{tb + 2}")
                ops.append(pf)
            return ops

        def emit_qT_mms(sb, xq, qT):
            """32 MM closures (jc-major); last per jc copies psum -> qT chunk
            jc (head A rows 0:64, head B rows 64:128 — natural layout)."""
            ops = []
            psq_box = {}

            def mk(jc, d):
                def op():
                    if d == 0:
                        psq_box[jc] = fppool.tile([128, c.S_BLK], f32, tag="fp",
                                                  name=f"psq{sb}_{jc}")
                    for h in range(2):
                        nc.tensor.matmul(
                            psq_box[jc][h * 64:(h + 1) * 64, :],
                            wq_sb[:, d * c.JW + jc * 128 + h * 64:
                                  d * c.JW + jc * 128 + (h + 1) * 64],
                            xq[:, d * c.S_BLK:(d + 1) * c.S_BLK],
                            start=(d == 0), stop=(d == c.DC - 1),
                            skip_group_check=True)
                    if d == c.DC - 1:
                        nc.vector.tensor_copy(
                            qT[:, jc * c.S_BLK:(jc + 1) * c.S_BLK],
                            psq_box[jc][:])
                return op
            for jc in range(c.JC):
                for d in range(c.DC):
                    ops.append(mk(jc, d))
            return ops

        def emit_outproj_mms(sb, catT):
            """Closures: per (sc, oc): 4 K=128 ic-MMs (each a co-running
            col-split half-pair) into a 1-bank psum, then copy + DMA out."""
            ops = []
            po_box = {}
            NIC = c.JC

            def mk(sc, oc, ic):
                def op():
                    if ic == 0:
                        po_box[(sc, oc)] = fppool.tile(
                            [128, c.OB], f32, tag="fp", name=f"po{sb}_{sc}_{oc}")
                    po = po_box[(sc, oc)]
                    for h in range(2):
                        nc.tensor.matmul(
                            po[h * 64:(h + 1) * 64, :],
                            catT[:, ic * c.S_BLK + sc * 128 + h * 64:
                                 ic * c.S_BLK + sc * 128 + (h + 1) * 64],
                            wo_sb[:, ic * c.D + oc * c.OB:
                                  ic * c.D + (oc + 1) * c.OB],
                            start=(ic == 0), stop=(ic == NIC - 1),
                            skip_group_check=True)
                    if ic == NIC - 1:
                        ot = opool.tile([128, c.OB], f32, tag="ot",
                                        name=f"ot{sb}_{sc}_{oc}")
                        nc.vector.tensor_copy(ot[:], po[:])
                        eng = nc.sync if (sc + oc) % 2 == 0 else nc.gpsimd
                        eng.dma_start(
                            out_d[sb * c.S_BLK + sc * 128:
                                  sb * c.S_BLK + (sc + 1) * 128,
                                  oc * c.OB:(oc + 1) * c.OB],
                            ot[:])
                return op
            for sc in range(c.S_BLK // 128):
                for oc in range(c.D // c.OB):
                    for ic in range(NIC):
                        ops.append(mk(sc, oc, ic))
            return ops

        # ---- slim prologue: only what unit (0,0) chunk 0 needs ----
        for op in kT_ops(0, 0):
            op()
        qT_tiles = {0: qpool.tile([128, c.JC * c.S_BLK], DT, tag="qT",
                                  name="qT0")}
        q0 = emit_qT_mms(0, xq_tiles[0], qT_tiles[0])
        for op in q0[0:8]:
            op()

        # ---- units + filler lists ----
        units = [(sb, hp) for sb in range(c.NSB) for hp in range(c.JC)]
        fillers = [[] for _ in units]
        # deadlines: kT(tb,jc) by chunk 4*tb of unit (0,jc); q0 jc by end of
        # unit (0,jc-1); v groups gated by v_done + deep PV lag.
        fillers[0] += kT_ops(1, 0) + kT_ops(2, 0) + kT_ops(3, 0) \
            + v_ops(0) + v_ops(1) + q0[8:16] + kT_ops(0, 1)
        fillers[1] += kT_ops(1, 1) + kT_ops(2, 1) + kT_ops(3, 1) \
            + v_ops(2) + v_ops(3) + q0[16:24] + kT_ops(0, 2)
        fillers[2] += kT_ops(1, 2) + kT_ops(2, 2) + kT_ops(3, 2) \
            + q0[24:32] + kT_ops(0, 3)
        fillers[3] += kT_ops(1, 3) + kT_ops(2, 3) + kT_ops(3, 3)

        cat_tiles = {}

        # ---- lagged-PV queue machinery (2 ops per t-chunk now) ----
        pvq = deque()        # (key, op, islast, need_v)
        stage_runs = {}      # key -> closure(cur_idx)
        pending_tail = []    # ops deferred past the last unit (drain)
        PVLAG = 64           # one full unit behind (4 ops/chunk)
        PVLAG0 = 64          # deep hold while sb-0's v' fillers land

        def emit_stage(sb, hp, catT, pv_state, cur_idx):
            """Normalize both heads from their pv psum tiles: reciprocal of
            the ones-row (psum[64:65]->sbuf[0:1], verified on HW), broadcast,
            one mul per head into catT block 2*hp+head. All ops stay on
            partitions 0:64 — no partition shifts."""
            X, Y = pv_state["X"], pv_state["Y"]
            # Free the psum banks FAST (three aligned copies, ~1.7us) so the
            # next unit's PV allocations don't stall the in-order PE queue
            # behind the slow reciprocal chain (3.3us per reciprocal).
            dn_sb = rpool.tile([128, c.S_BLK], f32, tag="dn",
                               name=f"dn{sb}_{hp}")
            nc.vector.tensor_copy(dn_sb[0:1, :], Y[0:1, :])
            nc.vector.tensor_copy(dn_sb[64:65, :], Y[64:65, :])
            stX = rpool.tile([128, c.S_BLK], f32, tag="stX",
                             name=f"stX{sb}_{hp}")
            nc.vector.tensor_copy(stX[:], X[:])
            for h in range(2):
                rti = rpool.tile([1, c.S_BLK], f32, tag=f"rti{h}",
                                 name=f"rti{h}_{sb}_{hp}")
                nc.vector.reciprocal(rti[:], dn_sb[h * 64:h * 64 + 1, :])
                rb = rpool.tile([128, c.S_BLK], f32, tag=f"rb{h}",
                                name=f"rb{h}_{sb}_{hp}")
                nc.gpsimd.partition_broadcast(rb[:], rti[:])
                nc.vector.tensor_mul(
                    catT[h * 64:(h + 1) * 64,
                         hp * c.S_BLK:(hp + 1) * c.S_BLK],
                    stX[h * 64:(h + 1) * 64, :], rb[h * 64:(h + 1) * 64, :])
            if hp == c.JC - 1:
                oops = emit_outproj_mms(sb, catT)
                splits = [(0, 12), (12, 24), (24, 32)]
                for j, (lo, hi) in enumerate(splits):
                    tgt = cur_idx + 1 + j
                    if tgt < len(units):
                        fillers[tgt] += oops[lo:hi]
                    else:
                        pending_tail.extend(oops[lo:hi])

        def make_pv_ops(key, sb, hp, t, es_tile, pv_state):
            """4 accumulating matmuls per t-chunk, in two co-running pairs:
            - value pair: head A -> X[0:64] (col group 0), head B ->
              X[64:128] (col group 64); K=128, same bank, concurrent.
            - denominator pair: ones-column lhsT, M=1: head A -> Y[0:1],
              head B -> Y[64:65]; concurrent col strips.
            Only the first matmul touching each bank carries start=True
            (start clears the whole bank)."""
            W = c.HL * c.VW
            first = (t == 0)
            last = (t == c.TCn - 1)

            def mkv(h):
                vsl = slice(t * W + (2 * hp + h) * c.VW,
                            t * W + (2 * hp + h) * c.VW + c.DK)
                ecol = slice(h * c.S_BLK, (h + 1) * c.S_BLK)

                def op():
                    if first and h == 0:
                        pv_state["X"] = pvpool.tile(
                            [128, c.S_BLK], f32, tag="pv",
                            name=f"pvX_{sb}_{hp}")
                    nc.tensor.matmul(
                        pv_state["X"][h * 64:(h + 1) * 64, :],
                        v_sb[:, vsl], es_tile[:, ecol],
                        start=first, stop=last,
                        skip_group_check=True)
                return op

            def mkd(h):
                osl = slice(t * W + (2 * hp + h) * c.VW + c.DK,
                            t * W + (2 * hp + h) * c.VW + c.VW)
                ecol = slice(h * c.S_BLK, (h + 1) * c.S_BLK)

                def op():
                    if first and h == 0:
                        pv_state["Y"] = pvpool.tile(
                            [128, c.S_BLK], f32, tag="pv",
                            name=f"pvY_{sb}_{hp}")
                    nc.tensor.matmul(
                        pv_state["Y"][h * 64:h * 64 + 1, :],
                        v_sb[:, osl], es_tile[:, ecol],
                        start=first, stop=last,
                        skip_group_check=True)
                return op
            need_v = t + 1 if sb == 0 else 0
            return [(key, mkv(0), False, need_v),
                    (key, mkv(1), False, need_v),
                    (key, mkd(0), False, need_v),
                    (key, mkd(1), last, need_v)]

        def pop_pv(n, cur_idx):
            for _ in range(n):
                if not pvq:
                    return
                if pvq[0][3] > v_done[0]:
                    return  # its v' chunk copy not yet emitted
                key, op, islast, _ = pvq.popleft()
                op()
                if islast:
                    stage_runs.pop(key)(cur_idx)

        # ---- main pipeline over units ----
        for idx, (sb, hp) in enumerate(units):
            if hp == 0:
                cat_tiles[sb] = cpool.tile([128, c.JC * c.S_BLK], DT,
                                           tag="cat", name=f"catT{sb}")
            if sb == 0 and hp == 3 and c.NSB > 2:
                xq_tiles[2] = load_x(xqpool, xqT, 2, c.S_BLK, "xq2")
            if sb == 0 and hp == 2 and c.NSB > 1:
                qT_tiles[1] = qpool.tile([128, c.JC * c.S_BLK], DT,
                                         tag="qT", name="qT1")
                q1 = emit_qT_mms(1, xq_tiles[1], qT_tiles[1])
                fillers[idx] += q1[:16]
                fillers[min(idx + 1, len(units) - 1)] += q1[16:]
            if sb >= 1 and hp == 0 and sb + 1 < c.NSB:
                if sb + 2 < c.NSB and sb + 2 not in xq_tiles:
                    def pfq(sb=sb):
                        xq_tiles[sb + 2] = load_x(xqpool, xqT, sb + 2,
                                                  c.S_BLK, f"xq{sb + 2}")
                    fillers[idx].append(pfq)
                qT_tiles[sb + 1] = qpool.tile([128, c.JC * c.S_BLK], DT,
                                              tag="qT", name=f"qT{sb + 1}")
                qops = emit_qT_mms(sb + 1, xq_tiles[sb + 1], qT_tiles[sb + 1])
                for j in range(4):
                    fillers[min(idx + j, len(units) - 1)] += qops[j * 8:(j + 1) * 8]
            catT = cat_tiles[sb]
            qT = qT_tiles[sb]
            key = (sb, hp)
            pv_state = {}
            stage_runs[key] = (
                lambda cur_idx, sb=sb, hp=hp, catT=catT, pv_state=pv_state:
                emit_stage(sb, hp, catT, pv_state, cur_idx))
            flist = fillers[idx]
            fpos = 0
            for t in range(c.TCn):
                if sb == 0 and hp <= 1:
                    lag = PVLAG0
                elif idx == len(units) - 1:
                    lag = max(12, PVLAG - 4 * t)
                else:
                    lag = PVLAG
                kcol = slice(hp * c.S + t * 128, hp * c.S + (t + 1) * 128)
                qcol = slice(hp * c.S_BLK, (hp + 1) * c.S_BLK)
                # fillers FIRST: the PE queue is in-order, so anything queued
                # behind a sem-waiting scores matmul stalls with it. Feeding
                # the quota up front keeps the PE busy through the exp wait
                # (and keeps its p-state high).
                want = (len(flist) * (t + 1)) // c.TCn
                while fpos < want:
                    flist[fpos]()
                    fpos += 1
                # backstop: if pops are v-gated and the queue nears the
                # es-ring capacity, pull fillers forward (advancing the v'
                # copies) and retry pops after each one
                while len(pvq) >= 72 and fpos < len(flist):
                    flist[fpos]()
                    fpos += 1
                    pop_pv(len(pvq) - lag, idx)
                pop_pv(len(pvq) - lag, idx)
                # One 2-bank psum tile [A(512) | B(512)] per t-chunk: head A
                # rows 0:64, head B rows 64:128 (concurrent row-split pair);
                # ONE exp covers both heads.
                ps2 = pspool.tile([128, 2 * c.S_BLK], f32, tag="ps",
                                  name=f"ps2_{sb}_{hp}_{t}")
                nc.tensor.matmul(
                    ps2[:, 0:c.S_BLK],
                    kT_sb[0:64, kcol], qT[0:64, qcol],
                    start=True, stop=True)
                nc.tensor.matmul(
                    ps2[:, c.S_BLK:2 * c.S_BLK],
                    kT_sb[64:128, kcol], qT[64:128, qcol],
                    start=True, stop=True)
                es_t = epool.tile([128, 2 * c.S_BLK], EXPDT, tag="exp",
                                  name=f"es{sb}_{hp}_{t}")
                nc.scalar.activation(
                    es_t[:], ps2[:], mybir.ActivationFunctionType.Exp,
                    scale=SCALE)
                pvq.extend(make_pv_ops(key, sb, hp, t, es_t, pv_state))
            while fpos < len(flist):
                flist[fpos]()
                fpos += 1
        # ---- drain ----
        pop_pv(len(pvq), len(units) - 1)
        for op in pending_tail:
            op()

    nc.compile()
    return nc


def shard_inputs(inputs: dict, cfg: Cfg, DT=mybir.dt.bfloat16):
    """Full inputs -> list of 8 per-core in_maps (numpy)."""
    npdt = DT_NP[DT]
    q, k, v = inputs["queries"], inputs["keys"], inputs["values"]
    Wq, Wk, Wv = inputs["Wq"], inputs["Wk"], inputs["Wv"]
    Wout = inputs["Wout"]
    B = q.shape[0]
    maps = []
    WoutT = np.ascontiguousarray(Wout.T)  # [i, o]
    for core in range(2 * B):
        b, half = divmod(core, 2)
        hs = slice(half * cfg.HL, (half + 1) * cfg.HL)
        i0 = half * cfg.JW
        maps.append({
            "xqT": np.ascontiguousarray(q[b].T).astype(npdt),
            "xkT": np.ascontiguousarray(k[b].T).astype(npdt),
            "xvT": np.ascontiguousarray(v[b].T).astype(npdt),
            "wq": np.ascontiguousarray(
                Wq[hs].transpose(1, 0, 2).reshape(cfg.D, cfg.JW)).astype(npdt),
            "wk": np.ascontiguousarray(
                Wk[hs].transpose(1, 0, 2).reshape(cfg.D, cfg.JW)).astype(npdt),
            "wv": np.ascontiguousarray(
                Wv[hs].transpose(1, 0, 2).reshape(cfg.D, cfg.JW)).astype(npdt),
            "woutT": np.ascontiguousarray(WoutT[i0:i0 + cfg.JW]).astype(npdt),
        })
    return maps


def gather_outputs(results, inputs):
    bout = inputs["bout"]
    B = inputs["queries"].shape[0]
    outs = []
    for b in range(B):
        outs.append(results[2 * b]["out"] + results[2 * b + 1]["out"] + bout)
    return np.stack(outs).astype(np.float32)


def percore_reference(in_map: dict, cfg: Cfg):
    """Numpy reference of what one core should produce (fp32 math)."""
    c = cfg
    xq = in_map["xqT"].astype(np.float32).T   # [S, D]
    xk = in_map["xkT"].astype(np.float32).T
    xv = in_map["xvT"].astype(np.float32).T
    wq = in_map["wq"].astype(np.float32)      # [D, JW]
    wk = in_map["wk"].astype(np.float32)
    wv = in_map["wv"].astype(np.float32)
    wo = in_map["woutT"].astype(np.float32)   # [JW, D]
    q = xq @ wq                               # [S, JW]
    k = xk @ wk
    v = xv @ wv
    cat = np.zeros((c.S, c.JW), dtype=np.float32)
    for h in range(c.HL):
        sl = slice(h * c.DK, (h + 1) * c.DK)
        s = (q[:, sl] @ k[:, sl].T) / np.sqrt(c.DK)
        e = np.exp(s)
        p = e / e.sum(axis=1, keepdims=True)
        cat[:, sl] = p @ v[:, sl]
    return cat @ wo

# ----------------------------------------------------------------------------
# Self-contained entry point: kernel(**inputs) -> full [B, S, D] output.
# ----------------------------------------------------------------------------
_NC_CACHE = {}


def _get_nc():
    key = "attn"
    if key not in _NC_CACHE:
        _NC_CACHE[key] = build_nc(Cfg(), mybir.dt.bfloat16, num_devices=8)
    return _NC_CACHE[key]


def kernel(**inputs):
    """Full (unsharded) inputs -> full [4, 2048, 1024] float32 output.

    Shards across the 8 NeuronCores as (batch x head-half), runs the Bass
    kernel SPMD, and gathers: out[b] = partial(core 2b) + partial(core 2b+1)
    + bias (row-sharded fc_out -> partial-sum reduction at gather time).
    """
    from concourse.bass_utils import run_bass_kernel_spmd

    inputs = {k: np.asarray(v) for k, v in inputs.items()}
    cfg = Cfg()
    nc = _get_nc()
    maps = shard_inputs(inputs, cfg, mybir.dt.bfloat16)
    res = run_bass_kernel_spmd(nc, maps, core_ids=list(range(8)), trace=False)
    return gather_outputs(res.results, inputs)
